# revision 19
# baseline (speedup 1.0000x reference)
"""Trainium2 Bass kernel for nn_Node2Property2 (segment_reduce), v2.

Model: out = segment_sum(softplus_shifted(x @ W1 + b1) @ W2, batch, G).

v2 strategy (8 cores, data-parallel over nodes; ~2x the v1 throughput):
  - x is shipped as fp8 e3m4 (2x pre-scale), W1 stationary as e3m4 (16x):
    halves input DMA vs bf16; mm1 runs fp8 with FWL.
  - Per 1024-node slot, v=W1q.T@xq lands in a [128,1024] f32 PSUM tile.
    "A" slots: ScalarE silu(C*v/32 + cb) -> bf16, the silu part of a
    softplus fit (AL*silu(C a+D) + GM*a + BE ~ softplus(a)).
    "P" slots: ONE custom DVE instruction evaluates an even cubic-in-t
    (t = min(v^2, T2)) fit of psi(a) = softplus(a) - a/2 straight from
    PSUM -> bf16. This offloads ~3/8 of the activation work from the
    (otherwise saturated) ScalarE to the Vector engine.
  - Both paths write h in a per-slot split layout [evens 512 | odds 512];
    a bf16 2x tensor_add forms pair sums ph (adjacent nodes share a graph
    except at segment boundaries -> fixed exactly on host).
  - mm2, A slots: one [H,16]-stationary bf16 matmul (w2 in col s) streams
    ph (512 pair cols) into a per-group [16,512] PSUM tile (rows 0:8
    used). P slots skip the DVE pairsum entirely: the poly op writes h as
    fp8 e4m3 and mm2 runs perf_mode=DoubleRow with (w2,w2) fp8 pairs
    (ko-stride 16B per s3_lw dual-fp8 rules), summing each node pair
    inside the PE. P rows are 8x-scaled (e4m3 subnormal dodge), divided
    out on host. A DVE copy + DMA per group evicts rows 0:8.
  - The host pre-permutes nodes within each slot (evens first, then
    odds) so every engine touches dense step-1 layouts - interleaved
    strided writes measured ~4x slower on ScalarE/DVE.
  - Host: segment-sum of pair sums; pairs straddling a segment boundary
    are recomputed exactly in f64; linear/constant folds (incl. the
    fp8-quantisation mean correction and the mm2-weight rounding
    correction) close the gap to softplus. Simulated rel err ~7.6e-3.

kernel(**inputs) takes FULL inputs, returns the FULL [G, 1] f32 output.
"""

import os
import sys

for _p in ("/opt/trn_rl_repo", "/root/.axon_site/_ro/trn_rl_repo"):
    if os.path.isdir(_p) and _p not in sys.path:
        sys.path.insert(0, _p)

import numpy as np
import ml_dtypes

import concourse.bacc as bacc
import concourse.mybir as mybir
import concourse.tile as tile
from concourse.bass_utils import run_bass_kernel_spmd

F32 = mybir.dt.float32
BF16 = mybir.dt.bfloat16
FP8E3 = mybir.dt.float8e3
FP8E4 = mybir.dt.float8e4
AF = mybir.ActivationFunctionType

E3 = ml_dtypes.float8_e3m4
E4 = ml_dtypes.float8_e4m3
BF = ml_dtypes.bfloat16

LOG2 = float(np.log(2.0))

# softplus(a) ~= AL*silu(C_*a + D_) + GM*a + BE  (v1's L2 fit, A-path)
AL = 1.16340907
C_ = 0.65158221
D_ = 6.08993352e-04
GM = 0.12077211
BE = 0.69315987

# Problem shape.
N, IN, H, OUT, G = 1048576, 128, 128, 1, 16384
NCORES = 8
NC_NODES = N // NCORES          # 131072 nodes per core

# Device tiling.
NS = 1024                       # nodes per slot
SLOTS = 8                       # slots per group
GRP = NS * SLOTS                # 8192 nodes per group
NGRP = NC_NODES // GRP          # 16 groups per core
PATTERN = os.environ.get("K_PATTERN", "AAPAAAPA")   # even groups
PATTERN2 = os.environ.get("K_PATTERN2", "AAPAAAPA")  # odd groups
K_EVICT = os.environ.get("K_EVICT", "dve")          # sps evict engine
K_PSUM = os.environ.get("K_PSUM", "slot")           # pairsum granularity
K_PMM2 = os.environ.get("K_PMM2", "dr8")            # P-slot mm2: dr8|bf16
XT_SPLIT = int(os.environ.get("K_XTS", "2"))    # input-DMA split per group
SC = 32.0                       # x*2 and W1*16 pre-scales (v_dev = 32 v)

# ---- P-path cubic fit: psi(u)=softplus(u)-u/2 ~ c0+c1 t+c2 t^2+c3 t^3,
#      t = min(u^2, T^2), Gaussian-weighted on [0, T] ----
_T = 6.5


def _fit_poly():
    u = np.linspace(0.0, _T, 40001)
    t = u * u
    psi = np.logaddexp(0.0, u) - 0.5 * u
    w = np.exp(-0.5 * u * u) + 3e-5
    A = np.stack([t**k for k in range(4)], axis=1)
    coef, *_ = np.linalg.lstsq(A * np.sqrt(w)[:, None], psi * np.sqrt(w),
                               rcond=None)
    return coef


_COEF = _fit_poly()
C0P = float(_COEF[0])
# device constants act on v_dev = SC*v (t_dev = v_dev^2), output /AL since
# the mm2 weights carry AL*W2
C1D = float(_COEF[1] / AL / SC**2)
C2D = float(_COEF[2] / AL / SC**4)
C3D = float(_COEF[3] / AL / SC**6)
T2D = float(_T * _T * SC * SC)

# ---- custom DVE op registration ----
from concourse.dve_spec import (Spec, Src0, C0, C1, C2, C3, minn, sq, lower,
                                _spill_c3_to_src1)
from concourse.dve_ops import (DveOp, OPS, CUSTOM_DVE_SPECS,
                               _SUB_OPCODE_FOR_NAME, _CUSTOM_DVE_ROW_BASE)
from concourse.dve_uop import DveOpSpec


def _register_poly_op():
    """out = ((s0*t + s1)*t + imm2)*t, t = min(sq(in0), C3 via in1)."""
    t = minn(sq(Src0), C3)
    body = _spill_c3_to_src1(((C0 * t + C1) * t + C2) * t)

    def ref(in0, in1, s0, s1, imm2):
        tt = np.minimum(np.float32(in0) * np.float32(in0), in1)
        return ((s0 * tt + s1) * tt + imm2) * tt

    op = DveOp.__new__(DveOp)
    object.__setattr__(op, "name", "SOFTPLUS_EVEN_P3")
    object.__setattr__(op, "spec", Spec(body=body, reference=ref))
    object.__setattr__(op, "subdim", False)
    object.__setattr__(op, "perf_en", {})
    shas = {}
    for ver in ("v3", "v4"):
        tmp = DveOpSpec(name=op.name, opcode=0, uops=lower(op.spec, ver=ver),
                        rd1_en=True)
        shas[ver] = tmp.sha(ver)
    object.__setattr__(op, "uops_sha", shas)
    if op.name not in _SUB_OPCODE_FOR_NAME:
        OPS.append(op)
        CUSTOM_DVE_SPECS[op.name] = op.spec
        _SUB_OPCODE_FOR_NAME[op.name] = _CUSTOM_DVE_ROW_BASE + len(OPS) - 1
    return op


_POLY = _register_poly_op()


def _build_nc(repeat=1):
    nc = bacc.Bacc("TRN2", target_bir_lowering=False, debug=False,
                   num_devices=NCORES)
    xT = nc.declare_dram_parameter("xT", [IN, NC_NODES], FP8E3, isOutput=False)
    W1 = nc.declare_dram_parameter("W1", [IN, H], FP8E3, isOutput=False)
    W2B = nc.declare_dram_parameter("W2B", [H, 128], BF16, isOutput=False)
    W2P = nc.declare_dram_parameter("W2P", [H, 256], FP8E4, isOutput=False)
    CB = nc.declare_dram_parameter("CB", [H, 1], F32, isOutput=False)
    T2T = nc.declare_dram_parameter("T2T", [H, 1], F32, isOutput=False)
    s_out = nc.declare_dram_parameter("s", [128, 512], F32, isOutput=True)

    with tile.TileContext(nc) as tc:
        with (
            tc.tile_pool(name="wts", bufs=1) as wts,
            tc.tile_pool(name="xp", bufs=3) as xp,
            tc.tile_pool(name="hp", bufs=int(os.environ.get("K_HP", "3"))) as hp,
            tc.tile_pool(name="php", bufs=2) as php,
            tc.tile_pool(name="h8p", bufs=3) as h8p,
            tc.tile_pool(name="stp", bufs=2) as stp,
            tc.tile_pool(name="vps", bufs=3, space="PSUM") as vps,
            tc.tile_pool(name="sps", bufs=2, space="PSUM") as sps,
        ):
            w1r = wts.tile([IN, H], FP8E3)
            w2r = wts.tile([H, 128], BF16)
            cbt = wts.tile([H, 1], F32)
            t2s = wts.tile([H, 1], F32)
            nc.sync.dma_start(w1r[:], W1[:])
            nc.sync.dma_start(w2r[:], W2B[:])
            nc.sync.dma_start(cbt[:], CB[:])
            nc.sync.dma_start(t2s[:], T2T[:])
            # stage weights via DVE so matmuls wait on one producer
            w1t = wts.tile([IN, H], FP8E3)
            nc.vector.tensor_copy(w1t[:], w1r[:])
            w2t = wts.tile([H, 128], BF16)
            nc.vector.tensor_copy(w2t[:], w2r[:])
            w2pr = wts.tile([H, 256], FP8E4)
            nc.sync.dma_start(w2pr[:], W2P[:])
            w2p = wts.tile([H, 256], FP8E4)
            nc.vector.tensor_copy(w2p[:], w2pr[:])
            # warm the silu table during the DMA ramp
            warm = wts.tile([H, 1], F32)
            nc.scalar.activation(warm[:], cbt[:], AF.Silu, bias=0.0, scale=1.0)

            state = {"spt": None}
            pending = []        # (g, s, ph_tile) mm2 not yet emitted

            def emit_mm2(n):
                for _ in range(n):
                    if not pending:
                        return
                    g, s, pht, kind = pending.pop(0)
                    if s == 0:
                        state["spt"] = sps.tile([16, 512], F32, name="spt")
                    spt = state["spt"]
                    if kind == "dr8":
                        nc.tensor.matmul(
                            spt[:],
                            w2p[:, 32 * s:32 * (s + 1)].rearrange(
                                "p (ko m) -> p ko m", ko=2),
                            pht[:].rearrange("p (ko m) -> p ko m", ko=2),
                            start=(s == 0), stop=(s == SLOTS - 1),
                            perf_mode=mybir.MatmulPerfMode.DoubleRow)
                    else:
                        nc.tensor.matmul(
                            spt[:], w2t[:, 16 * s:16 * s + 16],
                            pht[:, 512 * s:512 * (s + 1)],
                            start=(s == 0), stop=(s == SLOTS - 1))
                    if s == SLOTS - 1:
                        st = stp.tile([8, 512], F32, name="st")
                        nc.vector.tensor_copy(st[:], spt[0:8, :])
                        nc.sync.dma_start(
                            s_out[8 * (g % NGRP):8 * (g % NGRP) + 8, :],
                            st[:])

            for g_rep in range(repeat * NGRP):
                g = g_rep % NGRP
                gpat = PATTERN if g % 2 == 0 else PATTERN2
                xt = xp.tile([IN, GRP], FP8E3)
                qs = GRP // XT_SPLIT
                for q in range(XT_SPLIT):
                    nc.sync.dma_start(
                        xt[:, q * qs:(q + 1) * qs],
                        xT[:, g * GRP + q * qs:g * GRP + (q + 1) * qs])

                ht = hp.tile([H, GRP], BF16, name="ht")
                pht = php.tile([H, GRP // 2], BF16, name="pht")
                for s in range(SLOTS):
                    vt = vps.tile([H, NS], F32)
                    for k in range(2):
                        c0 = s * NS + k * 512
                        nc.tensor.matmul(vt[:, k * 512:(k + 1) * 512],
                                         w1t[:], xt[:, c0:c0 + 512],
                                         start=True, stop=True)
                    if gpat[s] == "A":
                        nc.scalar.activation(ht[:, s * NS:(s + 1) * NS],
                                             vt[:], AF.Silu,
                                             bias=cbt[:], scale=C_ / SC)
                        nc.vector.tensor_add(
                            pht[:, 512 * s:512 * (s + 1)],
                            ht[:, s * NS: s * NS + 512],
                            ht[:, s * NS + 512:(s + 1) * NS])
                        pending.append((g, s, pht, "bf16"))
                    elif K_PMM2 == "dr8":
                        h8 = h8p.tile([H, NS], FP8E4, name="h8")
                        nc.vector._custom_dve(_POLY, out=h8[:], in0=vt[:],
                                              in1=t2s[:], s0=C3D, s1=C2D,
                                              imm2=C1D)
                        pending.append((g, s, h8, "dr8"))
                    else:
                        nc.vector._custom_dve(_POLY,
                                              out=ht[:, s * NS:(s + 1) * NS],
                                              in0=vt[:], in1=t2s[:],
                                              s0=C3D, s1=C2D, imm2=C1D)
                        nc.vector.tensor_add(
                            pht[:, 512 * s:512 * (s + 1)],
                            ht[:, s * NS: s * NS + 512],
                            ht[:, s * NS + 512:(s + 1) * NS])
                        pending.append((g, s, pht, "bf16"))
                    # lag mm2 by one slot so the pairsum has landed
                    if s in (2, 5):
                        emit_mm2(3)
                emit_mm2(2)
            emit_mm2(len(pending))

    nc.compile()
    return nc


_NC_CACHE = {}


def _get_nc(repeat=1):
    if repeat not in _NC_CACHE:
        _NC_CACHE[repeat] = _build_nc(repeat)
    return _NC_CACHE[repeat]


def _prep_weights(W1, b1, W2):
    W1q = np.ascontiguousarray(
        np.clip(16.0 * W1.astype(np.float32), -15.5, 15.5).astype(E3))
    w2col = (AL * W2.astype(np.float64)).reshape(H)
    W2blk = np.zeros((H, 128), np.float64)
    for j in range(8):
        W2blk[:, 16 * j + j] = w2col
    W2blk = np.ascontiguousarray(W2blk.astype(BF))
    W2pblk = np.zeros((H, 256), np.float64)
    for s in range(8):
        for ko in range(2):
            W2pblk[:, 32 * s + 16 * ko + s] = 8.0 * w2col
    W2pblk = np.ascontiguousarray(W2pblk.astype(E4))
    cb = np.ascontiguousarray(
        (C_ * b1.astype(np.float64) + D_).astype(np.float32).reshape(H, 1))
    t2 = np.full((H, 1), T2D, np.float32)
    return W1q, W2blk, W2pblk, cb, t2


_PERM = None


def _slot_perm():
    """Within each 1024-node slot: evens first, then odds (dense pairsum)."""
    global _PERM
    if _PERM is None:
        idx = np.arange(NC_NODES).reshape(-1, NS)
        _PERM = np.concatenate([idx[:, 0::2], idx[:, 1::2]], axis=1).reshape(-1)
    return _PERM


def make_in_map(x_shard, W1, b1, W2):
    """Per-core input dict for one shard of nodes (helper for harnesses)."""
    W1q, W2blk, W2pblk, cb, t2 = _prep_weights(W1, b1, W2)
    xq = np.clip(2.0 * x_shard.astype(np.float32), -15.5, 15.5).astype(E3)
    xq = xq[_slot_perm()]
    return {
        "xT": np.ascontiguousarray(xq.T),
        "W1": W1q,
        "W2B": W2blk,
        "W2P": W2pblk,
        "CB": cb,
        "T2T": t2,
    }


def _run_device(x, W1, b1, W2):
    nc = _get_nc()
    in_maps = []
    for i in range(NCORES):
        sl = slice(i * NC_NODES, (i + 1) * NC_NODES)
        in_maps.append(make_in_map(x[sl], W1, b1, W2))
    res = run_bass_kernel_spmd(nc, in_maps, core_ids=list(range(NCORES)))
    # s[128,512] rows: 8*g + s_slot, cols: pair within slot -> natural order
    t_all = np.concatenate(
        [res.results[i]["s"].reshape(-1) for i in range(NCORES)])
    return t_all


def _node_is_poly():
    """[N] bool: which nodes went through the poly path (by position)."""
    pg_even = np.repeat(np.array([c == "P" for c in PATTERN]), NS)
    pg_odd = np.repeat(np.array([c == "P" for c in PATTERN2]), NS)
    two_groups = np.concatenate([pg_even, pg_odd])   # [2*GRP]
    return np.tile(two_groups, N // (2 * GRP))


def kernel(x, batch, W1, b1, W2, num_graphs):
    x = np.asarray(x)
    batch = np.asarray(batch).astype(np.int64, copy=False)
    W1 = np.asarray(W1)
    b1 = np.asarray(b1)
    W2 = np.asarray(W2)
    g_count = int(num_graphs)
    assert x.shape == (N, IN) and batch.shape == (N,)

    t_pair = _run_device(x, W1, b1, W2).astype(np.float64)

    W1d = W1.astype(np.float64)
    b1d = b1.astype(np.float64)
    W2d = W2.astype(np.float64).reshape(H)
    u_a = (AL * W2d).astype(BF).astype(np.float64) / AL
    if K_PMM2 == "dr8":
        u_p = (8.0 * AL * W2d).astype(E4).astype(np.float64) / 8.0 / AL
        tr = t_pair.reshape(NCORES, NGRP, 8, 512)
        for par, gsel in ((PATTERN, slice(0, NGRP, 2)),
                          (PATTERN2, slice(1, NGRP, 2))):
            p_slots = [i for i, c in enumerate(par) if c == "P"]
            tr[:, gsel, p_slots[0]:p_slots[0] + 1, :] /= 8.0
            for ps in p_slots[1:]:
                tr[:, gsel, ps:ps + 1, :] /= 8.0
        t_pair = tr.reshape(-1)
    else:
        u_p = u_a
    d2_a = W2d - u_a
    d2_p = W2d - u_p

    pf = batch[0::2]
    ps = batch[1::2]
    straddle = pf != ps
    out = np.zeros((g_count,), np.float64)
    ok = ~straddle
    np.add.at(out, pf[ok], t_pair[ok])
    idx_nodes = np.flatnonzero(np.repeat(straddle, 2))
    if idx_nodes.size:
        xs = x[idx_nodes].astype(np.float64)
        hs = np.logaddexp(0.0, xs @ W1d + b1d) - LOG2
        np.add.at(out, batch[idx_nodes], hs @ W2d)

    # ---- host folds ----
    xq = np.clip(2.0 * x.astype(np.float32), -15.5, 15.5).astype(E3)
    xq = xq.astype(np.float32) / 2.0
    W1q = np.clip(16.0 * W1.astype(np.float32), -15.5, 15.5).astype(E3)
    W1q = W1q.astype(np.float32) / 16.0
    xn = x.astype(np.float64)
    xqd = xq.astype(np.float64)
    W1qd = W1q.astype(np.float64)

    def lin(u, Wm, xm):
        return xm @ (Wm @ u)

    bu = lambda u: float(b1d @ u)
    ua_lin_q = lin(u_a, W1qd, xqd) + bu(u_a)
    ua_lin_t = lin(u_a, W1d, xn) + bu(u_a)
    up_lin_q = lin(u_p, W1qd, xqd) + bu(u_p)
    up_lin_t = lin(u_p, W1d, xn) + bu(u_p)
    d2a_lin = lin(d2_a, W1d, xn)
    d2p_lin = lin(d2_p, W1d, xn)
    b1_fold = 0.25 * lin(u_p * b1d, W1qd, xqd)

    sig = np.sqrt((W1qd ** 2).sum(axis=0))
    gh_x, gh_w = np.polynomial.hermite_e.hermegauss(40)
    ak = b1d[None, :] + sig[None, :] * gh_x[:, None]
    kappa0 = (np.logaddexp(0.0, ak) * (gh_w[:, None] / gh_w.sum())).sum(axis=0)
    d2a_const = float((d2_a * kappa0).sum())
    d2p_const = float((d2_p * kappa0).sum())

    sw_W2 = float(W2d.sum())

    node_poly = _node_is_poly()
    contrib = np.where(
        node_poly,
        0.5 * up_lin_q + C0P * float(u_p.sum()) + b1_fold
        + 0.5 * (up_lin_t - up_lin_q) + 0.5 * d2p_lin + d2p_const,
        GM * ua_lin_q + BE * float(u_a.sum())
        + 0.5 * (ua_lin_t - ua_lin_q) + 0.5 * d2a_lin + d2a_const)
    contrib = contrib - LOG2 * sw_W2
    contrib[idx_nodes] = 0.0
    np.add.at(out, batch, contrib)

    return out.astype(np.float32).reshape(g_count, OUT)
